# revision 30
# baseline (speedup 1.0000x reference)
"""Trainium2 Bass kernel for nn_Convs4x44 (dense_cnn, memory-bound).

Pipeline per sample (64 input floats -> 4 output floats):
  conv1 2x2/s2 on 8x8 -> relu(x-0.2) -> conv2 2x2/s2 on 4x4 -> relu(x-2)
  -> 4->3 linear + relu -> 3->2 linear -> softmax(2)

Strategy: pure data parallel over 8 cores. Sample-major SBUF layout
[128 partitions, C samples/partition * 64 feats] so both elementwise engines
run with all 128 lanes busy and the HBM loads are big contiguous blocks.

Every weighted-sum chain is normalized by its leading weight so the chain is
pure DVE scalar_tensor_tensor accumulation (out = in1 + scalar*in0) and the
leading weight + bias fold for free into the ACT activation that follows
(out = func(scale*x + bias)). DVE therefore runs only the unavoidable
2-input accumulates; ACT runs one activation per chain. softmax(2) is
computed exactly as sigmoid(+/-(raw0-raw1)). GPSIMD is deliberately idle:
its elementwise ops measured 3-20x slower and its SBUF port contention
slowed concurrent DVE ops ~3x.
"""

import os

import numpy as np

import concourse.bass as bass
import concourse.tile as tile
from concourse import mybir
from concourse.bass_utils import run_bass_kernel_spmd


def _split_multiwaits(nc):
    """This container's walrus build supports only ONE sync-wait command per
    instruction ("Too many sync wait commands" otherwise), while Tile freely
    emits multi-wait instructions. Split every instruction with N>1 waits
    into (N-1) same-engine NoOps carrying one wait each, inserted before it
    in the basic block; per-engine execution order is block order filtered
    by engine, so semantics are unchanged."""
    for func in nc.m.functions:
        for blk in func.blocks:
            insts = blk.instructions
            out = []
            changed = False
            for ins in insts:
                si = ins.sync_info
                if si is not None and len(si.on_wait) > 1:
                    waits = list(si.on_wait)
                    for k, w in enumerate(waits[:-1]):
                        nop = mybir.InstNoOp(
                            name=f"{ins.name}-wsplit-{k}", ins=[], outs=[])
                        nop.engine = ins.engine
                        nop.sync_info = mybir.SyncInfo(on_wait=[w], on_update=[])
                        out.append(nop)
                    ins.sync_info = mybir.SyncInfo(
                        on_wait=[waits[-1]], on_update=list(si.on_update))
                    changed = True
                out.append(ins)
            if changed:
                insts[:] = out


N_CORES = 8
B = 1048576
BC = B // N_CORES          # samples per core
P = 128                    # SBUF partitions
# per-tile samples-per-partition; sum * P == BC
TILE_CS = [224, 224, 224, 224, 128]
assert sum(TILE_CS) * P == BC

STORE_QUEUE = "scalar"
PE_MLP = True

F32 = mybir.dt.float32
ALU = mybir.AluOpType
AF = mybir.ActivationFunctionType

# columns in the broadcast-constant tile (exact coefficients; each chain
# opens with an ACT lead op carrying scale/bias so DVE runs only exact-
# coefficient scalar_tensor_tensor accumulates -- ratio-normalized chains
# measured the same speed but doubled rounding error on near-zero raws)
K1 = 0            # conv1_w taps [k00,k01,k10,k11]
K2 = 4            # conv2_w taps
W1C = 8           # W1[j,i] -> 8 + 4j + i
B1C = 20          # b1[j]
W2C = 23          # W2[j,i] -> 23 + 3j + i
B2C = 29          # b2[j]
SH1 = 31          # -0.2 (conv1 relu shift)
SH2 = 32          # -2.0 (f relu shift)
NW = 33


def _build(reps=1):
    nc = bass.Bass("TRN2", target_bir_lowering=False, debug=False,
                   num_devices=N_CORES)
    x = nc.dram_tensor("x", [BC, 64], F32, kind="ExternalInput")
    wconst = nc.dram_tensor("wconst", [P, NW], F32, kind="ExternalInput")
    # PE constants: cols 0:128 identity, 128:224 kron(I32,W1.T) [128,96],
    # 224:288 kron(I32,W2.T) [96,64] (rows 96: pad), 288 b1-pattern (96 rows),
    # 289 b2-pattern (64 rows)
    pconst = nc.dram_tensor("pconst", [P, 290], F32, kind="ExternalInput")
    out = nc.dram_tensor("out", [BC, 4], F32, kind="ExternalOutput")

    with tile.TileContext(nc) as tc:
        with (
            tc.tile_pool(name="consts", bufs=1) as cpool,
            tc.tile_pool(name="x", bufs=2) as xpool,
            tc.tile_pool(name="mid", bufs=2) as mpool,
            tc.tile_pool(name="small", bufs=2) as spool,
            tc.tile_pool(name="out", bufs=2) as opool,
            tc.tile_pool(name="tchain", bufs=1) as tpool,
            tc.psum_pool(name="ps", bufs=1) as ppool,
        ):
            ws = cpool.tile([P, NW], F32)
            nc.sync.dma_start(ws[:], wconst.ap()[:])
            pc0 = cpool.tile([P, 290], F32)
            nc.sync.dma_start(pc0[:], pconst.ap()[:])
            # PE reads constants from an ACT-written copy, not the DMA'd
            # tile: suspected cold-start DMA->PE dependency race on pass 1
            pc = cpool.tile([P, 290], F32)
            nc.scalar.activation(pc[:], pc0[:], AF.Copy, bias=0.0, scale=1.0)

            def sc(col):
                return ws[:, col:col + 1]

            CMAX = max(TILE_CS)
            s0 = 0
            pending = None
            for ti, C in enumerate(
                    [c for _ in range(reps) for c in TILE_CS]):
                if s0 >= BC:
                    s0 = 0
                # big loads on the idle SP ring; small stores on the ACT
                # ring (sharing the SP ring with loads cost ~9 us/pass,
                # loads on the ACT ring serialize behind activations)
                ldq = nc.sync
                stq = nc.scalar if STORE_QUEUE == "scalar" else nc.sync
                ns = P * C
                x_view = x.ap()[s0:s0 + ns, :].rearrange(
                    "(p c) f -> p (c f)", p=P, c=C)
                out_view = out.ap()[s0:s0 + ns, :].rearrange(
                    "(p c) four -> p (c four)", p=P, c=C)
                s0 += ns

                xt = xpool.tile([P, CMAX * 64], F32, tag="xt")
                ldq.dma_start(xt[:, :C * 64], x_view)

                # conv1: ACT lead (k00*A) then serial in-place exact-tap
                # accumulation; x1 = relu(t1 - 0.2) in place
                xv = xt[:, :C * 64].rearrange(
                    "p (c oh ti ow tj) -> p c oh ti ow tj", oh=4, ti=2, ow=4,
                    tj=2)
                t1 = mpool.tile([P, CMAX * 16], F32, tag="t1")
                t1v = t1[:, :C * 16].rearrange("p (c oh ow) -> p c oh ow",
                                               oh=4, ow=4)
                nc.scalar.activation(t1v, xv[:, :, :, 0, :, 0], AF.Copy,
                                     bias=0.0, scale=sc(K1 + 0))
                nc.vector.scalar_tensor_tensor(
                    t1v, xv[:, :, :, 0, :, 1], sc(K1 + 1), t1v,
                    ALU.mult, ALU.add)
                nc.vector.scalar_tensor_tensor(
                    t1v, xv[:, :, :, 1, :, 0], sc(K1 + 2), t1v,
                    ALU.mult, ALU.add)
                nc.vector.scalar_tensor_tensor(
                    t1v, xv[:, :, :, 1, :, 1], sc(K1 + 3), t1v,
                    ALU.mult, ALU.add)
                nc.scalar.activation(t1[:, :C * 16], t1[:, :C * 16], AF.Relu,
                                     bias=sc(SH1), scale=1.0)

                # conv2 on the 4x4 maps, same shape; f = relu(t2 - 2)
                x1v = t1[:, :C * 16].rearrange(
                    "p (c oh ti ow tj) -> p c oh ti ow tj", oh=2, ti=2, ow=2,
                    tj=2)
                t2 = spool.tile([P, CMAX * 4], F32, tag="t2")
                t2v = t2[:, :C * 4].rearrange("p (c oh ow) -> p c oh ow",
                                              oh=2, ow=2)
                nc.scalar.activation(t2v, x1v[:, :, :, 0, :, 0], AF.Copy,
                                     bias=0.0, scale=sc(K2 + 0))
                nc.vector.scalar_tensor_tensor(
                    t2v, x1v[:, :, :, 0, :, 1], sc(K2 + 1), t2v,
                    ALU.mult, ALU.add)
                nc.vector.scalar_tensor_tensor(
                    t2v, x1v[:, :, :, 1, :, 0], sc(K2 + 2), t2v,
                    ALU.mult, ALU.add)
                nc.vector.scalar_tensor_tensor(
                    t2v, x1v[:, :, :, 1, :, 1], sc(K2 + 3), t2v,
                    ALU.mult, ALU.add)
                nc.scalar.activation(t2[:, :C * 4], t2[:, :C * 4], AF.Relu,
                                     bias=sc(SH2), scale=1.0)
                fv = t2[:, :C * 4].rearrange("p (c i) -> p c i", i=4)

                # out tile layout per sample: [cls0, cls1, raw0, raw1]
                ot = opool.tile([P, CMAX * 4], F32, tag="ot")
                ov = ot[:, :C * 4].rearrange("p (c four) -> p c four", four=4)

                if PE_MLP:
                    # MLP on the tensor engine: transpose f into feature-major
                    # blocks, block-diag matmuls for W1/W2 (bias folded into
                    # the PSUM->SBUF activation), transpose raws back.
                    NF = C * 4
                    nblk = NF // 128
                    fT = ppool.tile([P, CMAX * 4], F32, tag="fT")
                    for k in range(nblk):
                        nc.tensor.matmul(
                            fT[:, k * 128:(k + 1) * 128],
                            t2[:, k * 128:(k + 1) * 128],
                            pc[:, 0:128], start=True, stop=True,
                            is_transpose=True)
                    fTs = tpool.tile([P, CMAX * 4], F32, tag="fTs")
                    nc.scalar.activation(fTs[:, :NF], fT[:, :NF], AF.Copy,
                                         bias=0.0, scale=1.0)
                    hp = ppool.tile([96, CMAX * 4], F32, tag="hp")
                    # matmul outputs must not straddle a 2KB PSUM bank:
                    # chunk at 512-fp32 boundaries (also the fp32 moving max)
                    for s0_ in range(0, NF, 512):
                        w = min(512, NF - s0_)
                        nc.tensor.matmul(
                            hp[:, s0_:s0_ + w], pc[:, 128:224],
                            fTs[:, s0_:s0_ + w], start=True, stop=True)
                    hTs = tpool.tile([96, CMAX * 4], F32, tag="hTs")
                    nc.scalar.activation(hTs[:, :NF], hp[:, :NF], AF.Relu,
                                         bias=pc[:96, 288:289], scale=1.0)
                    rp = ppool.tile([64, CMAX * 4], F32, tag="rp")
                    for s0_ in range(0, NF, 512):
                        w = min(512, NF - s0_)
                        nc.tensor.matmul(
                            rp[:, s0_:s0_ + w], pc[:96, 224:288],
                            hTs[:96, s0_:s0_ + w], start=True, stop=True)
                    rTs = tpool.tile([64, CMAX * 4], F32, tag="rTs")
                    nc.scalar.activation(rTs[:, :NF], rp[:, :NF], AF.Identity,
                                         bias=pc[:64, 289:290], scale=1.0)
                    rT = ppool.tile([P, CMAX * 2], F32, tag="rT")
                    for k in range(nblk):
                        nc.tensor.matmul(
                            rT[:, k * 64:(k + 1) * 64],
                            rTs[:64, k * 128:(k + 1) * 128],
                            pc[:64, 0:64], start=True, stop=True,
                            is_transpose=True)
                    rTv = rT[:, :C * 2].rearrange("p (c j) -> p c j", j=2)
                    nc.scalar.activation(ov[:, :, 2:4], rTv, AF.Copy,
                                         bias=0.0, scale=1.0)
                else:
                    # h_j = relu(W1[j,:] f + b1_j), j-major [3C], relu in place
                    h = spool.tile([P, CMAX * 3], F32, tag="h")
                    for j in range(3):
                        hj = h[:, j * C:(j + 1) * C]
                        nc.scalar.activation(hj, fv[:, :, 0], AF.Identity,
                                             bias=sc(B1C + j),
                                             scale=sc(W1C + 4 * j))
                        for i in range(1, 4):
                            nc.vector.scalar_tensor_tensor(
                                hj, fv[:, :, i], sc(W1C + 4 * j + i), hj,
                                ALU.mult, ALU.add)
                        nc.scalar.activation(hj, hj, AF.Relu, bias=0.0,
                                             scale=1.0)
                    hrv = h[:, :C * 3].rearrange("p (j c) -> p j c", j=3)
                    for j in range(2):
                        rj = ov[:, :, 2 + j]
                        nc.scalar.activation(rj, hrv[:, 0, :], AF.Identity,
                                             bias=sc(B2C + j),
                                             scale=sc(W2C + 3 * j))
                        for i in range(1, 3):
                            nc.vector.scalar_tensor_tensor(
                                rj, hrv[:, i, :], sc(W2C + 3 * j + i), rj,
                                ALU.mult, ALU.add)

                # softmax tail (d on DVE) deferred one tile so the DVE queue
                # never stalls waiting on the PE/ACT chain of its own tile
                def tail(ov=ov, C=C, ot=ot, out_view=out_view, stq=stq):
                    d = spool.tile([P, CMAX], F32, tag="d")
                    nc.vector.tensor_sub(d[:, :C], ov[:, :, 2], ov[:, :, 3])
                    nc.scalar.activation(ov[:, :, 0], d[:, :C], AF.Sigmoid,
                                         bias=0.0, scale=1.0)
                    nc.scalar.activation(ov[:, :, 1], d[:, :C], AF.Sigmoid,
                                         bias=0.0, scale=-1.0)
                    stq.dma_start(out_view, ot[:, :C * 4])

                if pending is not None:
                    pending()
                pending = tail

            if pending is not None:
                pending()

    _split_multiwaits(nc)
    return nc


_NC = None


def _get_nc():
    global _NC
    if _NC is None:
        _NC = _build()
    return _NC


def _pe_consts(W1, b1, W2, b2):
    W1 = np.asarray(W1, dtype=np.float32).reshape(3, 4)
    W2 = np.asarray(W2, dtype=np.float32).reshape(2, 3)
    b1 = np.asarray(b1, dtype=np.float32).reshape(3)
    b2 = np.asarray(b2, dtype=np.float32).reshape(2)
    pc = np.zeros((P, 290), dtype=np.float32)
    pc[:, 0:128] = np.eye(128, dtype=np.float32)
    pc[:, 128:224] = np.kron(np.eye(32, dtype=np.float32), W1.T)
    pc[:96, 224:288] = np.kron(np.eye(32, dtype=np.float32), W2.T)
    pc[:96, 288] = np.tile(b1, 32)
    pc[:64, 289] = np.tile(b2, 32)
    return np.ascontiguousarray(pc)


def _wconst_row(conv1_w, conv2_w, W1, b1, W2, b2):
    row = np.concatenate([
        np.asarray(conv1_w, dtype=np.float32).reshape(4),
        np.asarray(conv2_w, dtype=np.float32).reshape(4),
        np.asarray(W1, dtype=np.float32).reshape(12),
        np.asarray(b1, dtype=np.float32).reshape(3),
        np.asarray(W2, dtype=np.float32).reshape(6),
        np.asarray(b2, dtype=np.float32).reshape(2),
        np.array([-0.2, -2.0], dtype=np.float32),
    ])
    assert row.shape[0] == NW
    return row


TIMED_REPS = 32


def _timed(np_inputs, iters=16, reps=TIMED_REPS):
    """Measure steady-state per-pass HW time.

    Builds a timing variant of the kernel that repeats the full pipeline
    `reps` times inside one NEFF execution (re-reading the same HBM input),
    so device time per call (~reps * pass) dwarfs host dispatch (~1 ms) and
    the axon sync overhead (~75 ms) cancels in a two-burst slope. Calls are
    serialized by donation-chaining the output buffer.
    """
    import time

    import jax
    import jax.core
    import jax.numpy as jnp
    from jax.experimental.shard_map import shard_map
    from jax.sharding import Mesh, NamedSharding, PartitionSpec

    from concourse import bass2jax as b2j

    x = np.ascontiguousarray(
        np.asarray(np_inputs["x"], dtype=np.float32).reshape(B, 64))
    row = _wconst_row(np_inputs["conv1_w"], np_inputs["conv2_w"],
                      np_inputs["W1"], np_inputs["b1"], np_inputs["W2"],
                      np_inputs["b2"])
    wconst = np.ascontiguousarray(
        np.tile(row[None, :], (P * N_CORES, 1)).astype(np.float32))
    pcst = np.ascontiguousarray(np.tile(
        _pe_consts(np_inputs["W1"], np_inputs["b1"], np_inputs["W2"],
                   np_inputs["b2"]), (N_CORES, 1)))

    nc = _build(reps=reps)
    b2j.install_neuronx_cc_hook()
    devices = jax.devices()[:N_CORES]
    mesh = Mesh(np.asarray(devices), ("core",))
    spec = PartitionSpec("core")
    sh = NamedSharding(mesh, spec)
    out_aval = jax.core.ShapedArray((BC, 4), jnp.float32)

    def _body(xs, ws, ps, zs):
        outs = b2j._bass_exec_p.bind(
            xs, ws, ps, zs, b2j.partition_id_tensor(),
            out_avals=(out_aval,),
            in_names=("x", "wconst", "pconst", "out", "partition_id"),
            out_names=("out",),
            lowering_input_output_aliases=(),
            sim_require_finite=True,
            sim_require_nnan=True,
            nc=nc,
        )
        return outs[0]

    fn = jax.jit(
        shard_map(_body, mesh=mesh, in_specs=(spec, spec, spec, spec),
                  out_specs=spec, check_rep=False),
        donate_argnums=(3,), keep_unused=True)

    X = jax.device_put(x, sh)
    W = jax.device_put(wconst, sh)
    PC = jax.device_put(pcst, sh)
    X.block_until_ready()
    W.block_until_ready()
    PC.block_until_ready()

    z = fn(X, W, PC, np.zeros((B, 4), np.float32))
    z.block_until_ready()  # compile + warm

    def run_n(n, z):
        t0 = time.perf_counter()
        for _ in range(n):
            z = fn(X, W, PC, z)
        z.block_until_ready()
        return time.perf_counter() - t0, z

    base = 2
    slopes = []
    for _ in range(5):
        t1, z = run_n(base, z)
        t2, z = run_n(base + iters, z)
        slopes.append((t2 - t1) / iters)
    slopes.sort()
    if os.environ.get("TIMED_VERBOSE"):
        print("slopes/pass us:",
              [f"{s / reps * 1e6:.1f}" for s in slopes], flush=True)
    ns = slopes[len(slopes) // 2] / reps * 1e9
    return ns, np.asarray(z)


def kernel(x, conv1_w, conv2_w, W1, b1, W2, b2):
    x = np.ascontiguousarray(np.asarray(x, dtype=np.float32)).reshape(B, 64)
    row = _wconst_row(conv1_w, conv2_w, W1, b1, W2, b2)
    wconst = np.ascontiguousarray(np.tile(row[None, :], (P, 1)))

    nc = _get_nc()
    pcst = _pe_consts(W1, b1, W2, b2)
    in_maps = [
        {"x": np.ascontiguousarray(x[i * BC:(i + 1) * BC]), "wconst": wconst,
         "pconst": pcst}
        for i in range(N_CORES)
    ]
    res = run_bass_kernel_spmd(nc, in_maps, core_ids=list(range(N_CORES)))
    out = np.concatenate([res.results[i]["out"] for i in range(N_CORES)], axis=0)
    classification = np.ascontiguousarray(out[:, 0:2])
    raw = np.ascontiguousarray(out[:, 2:4])
    return classification, raw


# revision 33
# speedup vs baseline: 1.0907x; 1.0907x over previous
"""Trainium2 Bass kernel for nn_Convs4x44 (dense_cnn, memory-bound).

Pipeline per sample (64 input floats -> 4 output floats):
  conv1 2x2/s2 on 8x8 -> relu(x-0.2) -> conv2 2x2/s2 on 4x4 -> relu(x-2)
  -> 4->3 linear + relu -> 3->2 linear -> softmax(2)

Strategy: pure data parallel over 8 cores. Sample-major SBUF layout
[128 partitions, C samples/partition * 64 feats] so both elementwise engines
run with all 128 lanes busy and the HBM loads are big contiguous blocks.

Every weighted-sum chain is normalized by its leading weight so the chain is
pure DVE scalar_tensor_tensor accumulation (out = in1 + scalar*in0) and the
leading weight + bias fold for free into the ACT activation that follows
(out = func(scale*x + bias)). DVE therefore runs only the unavoidable
2-input accumulates; ACT runs one activation per chain. softmax(2) is
computed exactly as sigmoid(+/-(raw0-raw1)). GPSIMD is deliberately idle:
its elementwise ops measured 3-20x slower and its SBUF port contention
slowed concurrent DVE ops ~3x.
"""

import os

import numpy as np

import concourse.bass as bass
import concourse.tile as tile
from concourse import mybir
from concourse.bass_utils import run_bass_kernel_spmd


def _split_multiwaits(nc):
    """This container's walrus build supports only ONE sync-wait command per
    instruction ("Too many sync wait commands" otherwise), while Tile freely
    emits multi-wait instructions. Split every instruction with N>1 waits
    into (N-1) same-engine NoOps carrying one wait each, inserted before it
    in the basic block; per-engine execution order is block order filtered
    by engine, so semantics are unchanged."""
    for func in nc.m.functions:
        for blk in func.blocks:
            insts = blk.instructions
            out = []
            changed = False
            for ins in insts:
                si = ins.sync_info
                if si is not None and len(si.on_wait) > 1:
                    waits = list(si.on_wait)
                    for k, w in enumerate(waits[:-1]):
                        nop = mybir.InstNoOp(
                            name=f"{ins.name}-wsplit-{k}", ins=[], outs=[])
                        nop.engine = ins.engine
                        nop.sync_info = mybir.SyncInfo(on_wait=[w], on_update=[])
                        out.append(nop)
                    ins.sync_info = mybir.SyncInfo(
                        on_wait=[waits[-1]], on_update=list(si.on_update))
                    changed = True
                out.append(ins)
            if changed:
                insts[:] = out


N_CORES = 8
B = 1048576
BC = B // N_CORES          # samples per core
P = 128                    # SBUF partitions
# per-tile samples-per-partition; sum * P == BC
TILE_CS = [256, 256, 256, 256]
assert sum(TILE_CS) * P == BC

STORE_QUEUE = "scalar"
PE_MLP = True

F32 = mybir.dt.float32
ALU = mybir.AluOpType
AF = mybir.ActivationFunctionType

# columns in the broadcast-constant tile (exact coefficients; each chain
# opens with an ACT lead op carrying scale/bias so DVE runs only exact-
# coefficient scalar_tensor_tensor accumulates -- ratio-normalized chains
# measured the same speed but doubled rounding error on near-zero raws)
K1 = 0            # conv1_w taps [k00,k01,k10,k11]
K2 = 4            # conv2_w taps
W1C = 8           # W1[j,i] -> 8 + 4j + i
B1C = 20          # b1[j]
W2C = 23          # W2[j,i] -> 23 + 3j + i
B2C = 29          # b2[j]
SH1 = 31          # -0.2 (conv1 relu shift)
SH2 = 32          # -2.0 (f relu shift)
NW = 33


def _build(reps=1):
    nc = bass.Bass("TRN2", target_bir_lowering=False, debug=False,
                   num_devices=N_CORES)
    x = nc.dram_tensor("x", [BC, 64], F32, kind="ExternalInput")
    wconst = nc.dram_tensor("wconst", [P, NW], F32, kind="ExternalInput")
    # PE constants: cols 0:128 identity, 128:224 kron(I32,W1.T) [128,96],
    # 224:320 kron(I32,W2ext.T) [96,96] where W2ext rows are
    # [W2_0, W2_1, W2_0-W2_1] (the softmax diff computed on PE),
    # 320 b1-pattern (96 rows), 321 b2ext-pattern (96 rows)
    pconst = nc.dram_tensor("pconst", [P, 322], F32, kind="ExternalInput")
    out = nc.dram_tensor("out", [BC, 4], F32, kind="ExternalOutput")

    with tile.TileContext(nc) as tc:
        with (
            tc.tile_pool(name="consts", bufs=1) as cpool,
            tc.tile_pool(name="x", bufs=2) as xpool,
            tc.tile_pool(name="mid", bufs=2) as mpool,
            tc.tile_pool(name="small", bufs=2) as spool,
            tc.tile_pool(name="out", bufs=2) as opool,
            tc.tile_pool(name="tchain", bufs=1) as tpool,
            tc.psum_pool(name="ps", bufs=1) as ppool,
        ):
            ws = cpool.tile([P, NW], F32)
            nc.sync.dma_start(ws[:], wconst.ap()[:])
            pc0 = cpool.tile([P, 322], F32)
            nc.sync.dma_start(pc0[:], pconst.ap()[:])
            # PE reads constants from an ACT-written copy, not the DMA'd
            # tile: suspected cold-start DMA->PE dependency race on pass 1
            pc = cpool.tile([P, 322], F32)
            nc.scalar.activation(pc[:], pc0[:], AF.Copy, bias=0.0, scale=1.0)

            def sc(col):
                return ws[:, col:col + 1]

            CMAX = max(TILE_CS)
            s0 = 0
            for ti, C in enumerate(
                    [c for _ in range(reps) for c in TILE_CS]):
                if s0 >= BC:
                    s0 = 0
                # big loads on the idle SP ring; small stores on the ACT
                # ring (sharing the SP ring with loads cost ~9 us/pass,
                # loads on the ACT ring serialize behind activations)
                ldq = nc.sync
                stq = nc.scalar if STORE_QUEUE == "scalar" else nc.sync
                ns = P * C
                x_view = x.ap()[s0:s0 + ns, :].rearrange(
                    "(p c) f -> p (c f)", p=P, c=C)
                out_view = out.ap()[s0:s0 + ns, :].rearrange(
                    "(p c) four -> p (c four)", p=P, c=C)
                s0 += ns

                xt = xpool.tile([P, CMAX * 64], F32, tag="xt")
                ldq.dma_start(xt[:, :C * 64], x_view)

                # conv1: ACT lead (k00*A) then serial in-place exact-tap
                # accumulation; x1 = relu(t1 - 0.2) in place
                xv = xt[:, :C * 64].rearrange(
                    "p (c oh ti ow tj) -> p c oh ti ow tj", oh=4, ti=2, ow=4,
                    tj=2)
                t1 = mpool.tile([P, CMAX * 16], F32, tag="t1")
                t1v = t1[:, :C * 16].rearrange("p (c oh ow) -> p c oh ow",
                                               oh=4, ow=4)
                nc.scalar.activation(t1v, xv[:, :, :, 0, :, 0], AF.Copy,
                                     bias=0.0, scale=sc(K1 + 0))
                nc.vector.scalar_tensor_tensor(
                    t1v, xv[:, :, :, 0, :, 1], sc(K1 + 1), t1v,
                    ALU.mult, ALU.add)
                nc.vector.scalar_tensor_tensor(
                    t1v, xv[:, :, :, 1, :, 0], sc(K1 + 2), t1v,
                    ALU.mult, ALU.add)
                nc.vector.scalar_tensor_tensor(
                    t1v, xv[:, :, :, 1, :, 1], sc(K1 + 3), t1v,
                    ALU.mult, ALU.add)
                nc.scalar.activation(t1[:, :C * 16], t1[:, :C * 16], AF.Relu,
                                     bias=sc(SH1), scale=1.0)

                # conv2 on the 4x4 maps, same shape; f = relu(t2 - 2)
                x1v = t1[:, :C * 16].rearrange(
                    "p (c oh ti ow tj) -> p c oh ti ow tj", oh=2, ti=2, ow=2,
                    tj=2)
                t2 = spool.tile([P, CMAX * 4], F32, tag="t2")
                t2v = t2[:, :C * 4].rearrange("p (c oh ow) -> p c oh ow",
                                              oh=2, ow=2)
                nc.scalar.activation(t2v, x1v[:, :, :, 0, :, 0], AF.Copy,
                                     bias=0.0, scale=sc(K2 + 0))
                nc.vector.scalar_tensor_tensor(
                    t2v, x1v[:, :, :, 0, :, 1], sc(K2 + 1), t2v,
                    ALU.mult, ALU.add)
                nc.vector.scalar_tensor_tensor(
                    t2v, x1v[:, :, :, 1, :, 0], sc(K2 + 2), t2v,
                    ALU.mult, ALU.add)
                nc.vector.scalar_tensor_tensor(
                    t2v, x1v[:, :, :, 1, :, 1], sc(K2 + 3), t2v,
                    ALU.mult, ALU.add)
                nc.scalar.activation(t2[:, :C * 4], t2[:, :C * 4], AF.Relu,
                                     bias=sc(SH2), scale=1.0)
                fv = t2[:, :C * 4].rearrange("p (c i) -> p c i", i=4)

                # out tile layout per sample: [cls0, cls1, raw0, raw1]
                ot = opool.tile([P, CMAX * 4], F32, tag="ot")
                ov = ot[:, :C * 4].rearrange("p (c four) -> p c four", four=4)

                if PE_MLP:
                    # MLP on the tensor engine: transpose f into feature-major
                    # blocks, block-diag matmuls for W1/W2ext (bias folded
                    # into the PSUM->SBUF activation; W2ext also emits
                    # d = raw0-raw1 per sample), transpose back.
                    NF = C * 4
                    nblk = NF // 128
                    fT = ppool.tile([P, CMAX * 4], F32, tag="fT")
                    for k in range(nblk):
                        nc.tensor.matmul(
                            fT[:, k * 128:(k + 1) * 128],
                            t2[:, k * 128:(k + 1) * 128],
                            pc[:, 0:128], start=True, stop=True,
                            is_transpose=True)
                    fTs = tpool.tile([P, CMAX * 4], F32, tag="fTs")
                    nc.scalar.activation(fTs[:, :NF], fT[:, :NF], AF.Copy,
                                         bias=0.0, scale=1.0)
                    hp = ppool.tile([96, CMAX * 4], F32, tag="hp")
                    # matmul outputs must not straddle a 2KB PSUM bank:
                    # chunk at 512-fp32 boundaries (also the fp32 moving max)
                    for s0_ in range(0, NF, 512):
                        w = min(512, NF - s0_)
                        nc.tensor.matmul(
                            hp[:, s0_:s0_ + w], pc[:, 128:224],
                            fTs[:, s0_:s0_ + w], start=True, stop=True)
                    hTs = tpool.tile([96, CMAX * 4], F32, tag="hTs")
                    nc.scalar.activation(hTs[:, :NF], hp[:, :NF], AF.Relu,
                                         bias=pc[:96, 320:321], scale=1.0)
                    rp = ppool.tile([96, CMAX * 4], F32, tag="rp")
                    for s0_ in range(0, NF, 512):
                        w = min(512, NF - s0_)
                        nc.tensor.matmul(
                            rp[:, s0_:s0_ + w], pc[:96, 224:320],
                            hTs[:96, s0_:s0_ + w], start=True, stop=True)
                    rTs = tpool.tile([96, CMAX * 4], F32, tag="rTs")
                    nc.scalar.activation(rTs[:, :NF], rp[:, :NF], AF.Identity,
                                         bias=pc[:96, 321:322], scale=1.0)
                    # back-transpose: each [128, 96] chunk parked in a 512B-
                    # aligned 128-col slot so no output straddles a bank
                    rT = ppool.tile([P, CMAX * 4], F32, tag="rT")
                    for k in range(nblk):
                        nc.tensor.matmul(
                            rT[:, k * 128:k * 128 + 96],
                            rTs[:96, k * 128:(k + 1) * 128],
                            pc[:96, 0:96], start=True, stop=True,
                            is_transpose=True)
                    rTq = rT[:, :nblk * 128].rearrange(
                        "p (k w) -> p k w", w=128)[:, :, 0:96].rearrange(
                        "p k (s j) -> p k s j", j=3)
                    ovq = ov[:, :, 2:4].rearrange(
                        "p (k s) j -> p k s j", k=nblk)
                    nc.scalar.activation(ovq, rTq[:, :, :, 0:2], AF.Copy,
                                         bias=0.0, scale=1.0)
                    cls0 = ov[:, :, 0].rearrange("p (k s) -> p k s", k=nblk)
                    cls1 = ov[:, :, 1].rearrange("p (k s) -> p k s", k=nblk)
                    dv = rTq[:, :, :, 2]
                    nc.scalar.activation(cls0, dv, AF.Sigmoid,
                                         bias=0.0, scale=1.0)
                    nc.scalar.activation(cls1, dv, AF.Sigmoid,
                                         bias=0.0, scale=-1.0)
                    stq.dma_start(out_view, ot[:, :C * 4])
                else:
                    # h_j = relu(W1[j,:] f + b1_j), j-major [3C], relu in place
                    h = spool.tile([P, CMAX * 3], F32, tag="h")
                    for j in range(3):
                        hj = h[:, j * C:(j + 1) * C]
                        nc.scalar.activation(hj, fv[:, :, 0], AF.Identity,
                                             bias=sc(B1C + j),
                                             scale=sc(W1C + 4 * j))
                        for i in range(1, 4):
                            nc.vector.scalar_tensor_tensor(
                                hj, fv[:, :, i], sc(W1C + 4 * j + i), hj,
                                ALU.mult, ALU.add)
                        nc.scalar.activation(hj, hj, AF.Relu, bias=0.0,
                                             scale=1.0)
                    hrv = h[:, :C * 3].rearrange("p (j c) -> p j c", j=3)
                    for j in range(2):
                        rj = ov[:, :, 2 + j]
                        nc.scalar.activation(rj, hrv[:, 0, :], AF.Identity,
                                             bias=sc(B2C + j),
                                             scale=sc(W2C + 3 * j))
                        for i in range(1, 3):
                            nc.vector.scalar_tensor_tensor(
                                rj, hrv[:, i, :], sc(W2C + 3 * j + i), rj,
                                ALU.mult, ALU.add)

                if not PE_MLP:
                    d = spool.tile([P, CMAX], F32, tag="d")
                    nc.vector.tensor_sub(d[:, :C], ov[:, :, 2], ov[:, :, 3])
                    nc.scalar.activation(ov[:, :, 0], d[:, :C], AF.Sigmoid,
                                         bias=0.0, scale=1.0)
                    nc.scalar.activation(ov[:, :, 1], d[:, :C], AF.Sigmoid,
                                         bias=0.0, scale=-1.0)
                    stq.dma_start(out_view, ot[:, :C * 4])

    _split_multiwaits(nc)
    return nc


_NC = None


def _get_nc():
    global _NC
    if _NC is None:
        _NC = _build()
    return _NC


def _pe_consts(W1, b1, W2, b2):
    W1 = np.asarray(W1, dtype=np.float32).reshape(3, 4)
    W2 = np.asarray(W2, dtype=np.float32).reshape(2, 3)
    b1 = np.asarray(b1, dtype=np.float32).reshape(3)
    b2 = np.asarray(b2, dtype=np.float32).reshape(2)
    W2e = np.vstack([W2, W2[0] - W2[1]])                # [3, 3]
    b2e = np.array([b2[0], b2[1], b2[0] - b2[1]], dtype=np.float32)
    pc = np.zeros((P, 322), dtype=np.float32)
    pc[:, 0:128] = np.eye(128, dtype=np.float32)
    pc[:, 128:224] = np.kron(np.eye(32, dtype=np.float32), W1.T)
    pc[:96, 224:320] = np.kron(np.eye(32, dtype=np.float32), W2e.T)
    pc[:96, 320] = np.tile(b1, 32)
    pc[:96, 321] = np.tile(b2e, 32)
    return np.ascontiguousarray(pc)


def _wconst_row(conv1_w, conv2_w, W1, b1, W2, b2):
    row = np.concatenate([
        np.asarray(conv1_w, dtype=np.float32).reshape(4),
        np.asarray(conv2_w, dtype=np.float32).reshape(4),
        np.asarray(W1, dtype=np.float32).reshape(12),
        np.asarray(b1, dtype=np.float32).reshape(3),
        np.asarray(W2, dtype=np.float32).reshape(6),
        np.asarray(b2, dtype=np.float32).reshape(2),
        np.array([-0.2, -2.0], dtype=np.float32),
    ])
    assert row.shape[0] == NW
    return row


TIMED_REPS = 32


def _timed(np_inputs, iters=16, reps=TIMED_REPS):
    """Measure steady-state per-pass HW time.

    Builds a timing variant of the kernel that repeats the full pipeline
    `reps` times inside one NEFF execution (re-reading the same HBM input),
    so device time per call (~reps * pass) dwarfs host dispatch (~1 ms) and
    the axon sync overhead (~75 ms) cancels in a two-burst slope. Calls are
    serialized by donation-chaining the output buffer.
    """
    import time

    import jax
    import jax.core
    import jax.numpy as jnp
    from jax.experimental.shard_map import shard_map
    from jax.sharding import Mesh, NamedSharding, PartitionSpec

    from concourse import bass2jax as b2j

    x = np.ascontiguousarray(
        np.asarray(np_inputs["x"], dtype=np.float32).reshape(B, 64))
    row = _wconst_row(np_inputs["conv1_w"], np_inputs["conv2_w"],
                      np_inputs["W1"], np_inputs["b1"], np_inputs["W2"],
                      np_inputs["b2"])
    wconst = np.ascontiguousarray(
        np.tile(row[None, :], (P * N_CORES, 1)).astype(np.float32))
    pcst = np.ascontiguousarray(np.tile(
        _pe_consts(np_inputs["W1"], np_inputs["b1"], np_inputs["W2"],
                   np_inputs["b2"]), (N_CORES, 1)))

    nc = _build(reps=reps)
    b2j.install_neuronx_cc_hook()
    devices = jax.devices()[:N_CORES]
    mesh = Mesh(np.asarray(devices), ("core",))
    spec = PartitionSpec("core")
    sh = NamedSharding(mesh, spec)
    out_aval = jax.core.ShapedArray((BC, 4), jnp.float32)

    def _body(xs, ws, ps, zs):
        outs = b2j._bass_exec_p.bind(
            xs, ws, ps, zs, b2j.partition_id_tensor(),
            out_avals=(out_aval,),
            in_names=("x", "wconst", "pconst", "out", "partition_id"),
            out_names=("out",),
            lowering_input_output_aliases=(),
            sim_require_finite=True,
            sim_require_nnan=True,
            nc=nc,
        )
        return outs[0]

    fn = jax.jit(
        shard_map(_body, mesh=mesh, in_specs=(spec, spec, spec, spec),
                  out_specs=spec, check_rep=False),
        donate_argnums=(3,), keep_unused=True)

    X = jax.device_put(x, sh)
    W = jax.device_put(wconst, sh)
    PC = jax.device_put(pcst, sh)
    X.block_until_ready()
    W.block_until_ready()
    PC.block_until_ready()

    z = fn(X, W, PC, np.zeros((B, 4), np.float32))
    z.block_until_ready()  # compile + warm

    def run_n(n, z):
        t0 = time.perf_counter()
        for _ in range(n):
            z = fn(X, W, PC, z)
        z.block_until_ready()
        return time.perf_counter() - t0, z

    base = 2
    slopes = []
    for _ in range(5):
        t1, z = run_n(base, z)
        t2, z = run_n(base + iters, z)
        slopes.append((t2 - t1) / iters)
    slopes.sort()
    if os.environ.get("TIMED_VERBOSE"):
        print("slopes/pass us:",
              [f"{s / reps * 1e6:.1f}" for s in slopes], flush=True)
    ns = slopes[len(slopes) // 2] / reps * 1e9
    return ns, np.asarray(z)


def kernel(x, conv1_w, conv2_w, W1, b1, W2, b2):
    x = np.ascontiguousarray(np.asarray(x, dtype=np.float32)).reshape(B, 64)
    row = _wconst_row(conv1_w, conv2_w, W1, b1, W2, b2)
    wconst = np.ascontiguousarray(np.tile(row[None, :], (P, 1)))

    nc = _get_nc()
    pcst = _pe_consts(W1, b1, W2, b2)
    in_maps = [
        {"x": np.ascontiguousarray(x[i * BC:(i + 1) * BC]), "wconst": wconst,
         "pconst": pcst}
        for i in range(N_CORES)
    ]
    res = run_bass_kernel_spmd(nc, in_maps, core_ids=list(range(N_CORES)))
    out = np.concatenate([res.results[i]["out"] for i in range(N_CORES)], axis=0)
    classification = np.ascontiguousarray(out[:, 0:2])
    raw = np.ascontiguousarray(out[:, 2:4])
    return classification, raw


# revision 34
# speedup vs baseline: 1.1102x; 1.0179x over previous
"""Trainium2 Bass kernel for nn_Convs4x44 (dense_cnn, memory-bound).

Pipeline per sample (64 input floats -> 4 output floats):
  conv1 2x2/s2 on 8x8 -> relu(x-0.2) -> conv2 2x2/s2 on 4x4 -> relu(x-2)
  -> 4->3 linear + relu -> 3->2 linear -> softmax(2)

Strategy: pure data parallel over 8 cores; per core 4 tiles of 32K samples
in sample-major SBUF layout [128 partitions, C*64 feats], C=256. Measured
per-pass steady state ~113 us/core vs a ~108 us pure-DMA floor (33.5 MB in
+ 2 MB out at ~330 GB/s effective).

Engine assignment (all fp32; the rel-err gate demands ~1e-6 abs accuracy
on raw because it crosses zero, so no bf16 anywhere):
- DMA: x loads on the SP HWDGE ring, out stores on the ACT ring. Sharing
  one ring cost ~9 us/pass; loads on the ACT ring serialize behind
  activations (much worse).
- DVE: only the 6 unavoidable 2-input conv accumulates per tile - serial
  in-place exact-coefficient scalar_tensor_tensor chains (ACT opens each
  chain with a scaled-copy lead). Keeping DVE small matters: DVE and DMA
  contend for SBUF ports, and the contention scales with DVE busy time.
- ACT: chain leads, shifted relus (in place), and the PSUM<->SBUF copies
  of the PE chain with W-biases folded in via per-partition bias columns.
- PE (tensor engine): the whole MLP. Per tile: transpose f to
  feature-major PSUM blocks (fp32 PE transpose is exact), block-diagonal
  kron(I32, W1.T) matmul, relu-copy to SBUF, kron(I32, W2ext.T) matmul
  where W2ext = [W2_0; W2_1; W2_0-W2_1] also emits the softmax difference
  d per sample, then transpose back; sigmoids read the d lane straight
  from PSUM. PE fp32 matmul measured full fp32 precision (~5e-7).

Hardware rules learned the hard way (violations crash or corrupt):
- A matmul output must not straddle a 2KB PSUM bank: chunk the moving dim
  at 512-fp32 boundaries; park [128,96] transpose outputs in 512B slots.
- DMA->PE dependencies mis-sync on cold start: PE must read constants
  from an ACT-written copy (pc), not the DMA-landed tile (pc0).
- Keep SBUF usage <= ~195KB/partition (device hard-crashed at ~206KB).
- This walrus build allows only one sync-wait per instruction: see
  _split_multiwaits.
GPSIMD is deliberately idle (3-20x slower, SBUF port contention).
"""

import os

import numpy as np

import concourse.bass as bass
import concourse.tile as tile
from concourse import mybir
from concourse.bass_utils import run_bass_kernel_spmd


def _split_multiwaits(nc):
    """This container's walrus build supports only ONE sync-wait command per
    instruction ("Too many sync wait commands" otherwise), while Tile freely
    emits multi-wait instructions. Split every instruction with N>1 waits
    into (N-1) same-engine NoOps carrying one wait each, inserted before it
    in the basic block; per-engine execution order is block order filtered
    by engine, so semantics are unchanged."""
    for func in nc.m.functions:
        for blk in func.blocks:
            insts = blk.instructions
            out = []
            changed = False
            for ins in insts:
                si = ins.sync_info
                if si is not None and len(si.on_wait) > 1:
                    waits = list(si.on_wait)
                    for k, w in enumerate(waits[:-1]):
                        nop = mybir.InstNoOp(
                            name=f"{ins.name}-wsplit-{k}", ins=[], outs=[])
                        nop.engine = ins.engine
                        nop.sync_info = mybir.SyncInfo(on_wait=[w], on_update=[])
                        out.append(nop)
                    ins.sync_info = mybir.SyncInfo(
                        on_wait=[waits[-1]], on_update=list(si.on_update))
                    changed = True
                out.append(ins)
            if changed:
                insts[:] = out


N_CORES = 8
B = 1048576
BC = B // N_CORES          # samples per core
P = 128                    # SBUF partitions
# per-tile samples-per-partition; sum * P == BC
TILE_CS = [256, 256, 256, 256]
assert sum(TILE_CS) * P == BC

STORE_QUEUE = "scalar"
PE_MLP = True

F32 = mybir.dt.float32
ALU = mybir.AluOpType
AF = mybir.ActivationFunctionType

# columns in the broadcast-constant tile (exact coefficients; each chain
# opens with an ACT lead op carrying scale/bias so DVE runs only exact-
# coefficient scalar_tensor_tensor accumulates -- ratio-normalized chains
# measured the same speed but doubled rounding error on near-zero raws)
K1 = 0            # conv1_w taps [k00,k01,k10,k11]
K2 = 4            # conv2_w taps
W1C = 8           # W1[j,i] -> 8 + 4j + i
B1C = 20          # b1[j]
W2C = 23          # W2[j,i] -> 23 + 3j + i
B2C = 29          # b2[j]
SH1 = 31          # -0.2 (conv1 relu shift)
SH2 = 32          # -2.0 (f relu shift)
NW = 33


def _build(reps=1):
    nc = bass.Bass("TRN2", target_bir_lowering=False, debug=False,
                   num_devices=N_CORES)
    x = nc.dram_tensor("x", [BC, 64], F32, kind="ExternalInput")
    wconst = nc.dram_tensor("wconst", [P, NW], F32, kind="ExternalInput")
    # PE constants: cols 0:128 identity, 128:224 kron(I32,W1.T) [128,96],
    # 224:320 kron(I32,W2ext.T) [96,96] where W2ext rows are
    # [W2_0, W2_1, W2_0-W2_1] (the softmax diff computed on PE),
    # 320 b1-pattern (96 rows), 321 b2ext-pattern (96 rows)
    pconst = nc.dram_tensor("pconst", [P, 322], F32, kind="ExternalInput")
    out = nc.dram_tensor("out", [BC, 4], F32, kind="ExternalOutput")

    with tile.TileContext(nc) as tc:
        with (
            tc.tile_pool(name="consts", bufs=1) as cpool,
            tc.tile_pool(name="x", bufs=2) as xpool,
            tc.tile_pool(name="mid", bufs=2) as mpool,
            tc.tile_pool(name="small", bufs=2) as spool,
            tc.tile_pool(name="out", bufs=2) as opool,
            tc.tile_pool(name="tchain", bufs=1) as tpool,
            tc.psum_pool(name="ps", bufs=1) as ppool,
        ):
            ws = cpool.tile([P, NW], F32)
            nc.sync.dma_start(ws[:], wconst.ap()[:])
            pc0 = cpool.tile([P, 322], F32)
            nc.sync.dma_start(pc0[:], pconst.ap()[:])
            # PE reads constants from an ACT-written copy, not the DMA'd
            # tile: suspected cold-start DMA->PE dependency race on pass 1
            pc = cpool.tile([P, 322], F32)
            nc.scalar.activation(pc[:], pc0[:], AF.Copy, bias=0.0, scale=1.0)

            def sc(col):
                return ws[:, col:col + 1]

            CMAX = max(TILE_CS)
            s0 = 0
            for ti, C in enumerate(
                    [c for _ in range(reps) for c in TILE_CS]):
                if s0 >= BC:
                    s0 = 0
                # big loads on the idle SP ring; small stores on the ACT
                # ring (sharing the SP ring with loads cost ~9 us/pass,
                # loads on the ACT ring serialize behind activations)
                ldq = nc.sync
                stq = nc.scalar if STORE_QUEUE == "scalar" else nc.sync
                ns = P * C
                x_view = x.ap()[s0:s0 + ns, :].rearrange(
                    "(p c) f -> p (c f)", p=P, c=C)
                out_view = out.ap()[s0:s0 + ns, :].rearrange(
                    "(p c) four -> p (c four)", p=P, c=C)
                s0 += ns

                xt = xpool.tile([P, CMAX * 64], F32, tag="xt")
                ldq.dma_start(xt[:, :C * 64], x_view)

                # conv1: ACT lead (k00*A) then serial in-place exact-tap
                # accumulation; x1 = relu(t1 - 0.2) in place
                xv = xt[:, :C * 64].rearrange(
                    "p (c oh ti ow tj) -> p c oh ti ow tj", oh=4, ti=2, ow=4,
                    tj=2)
                t1 = mpool.tile([P, CMAX * 16], F32, tag="t1")
                t1v = t1[:, :C * 16].rearrange("p (c oh ow) -> p c oh ow",
                                               oh=4, ow=4)
                nc.scalar.activation(t1v, xv[:, :, :, 0, :, 0], AF.Copy,
                                     bias=0.0, scale=sc(K1 + 0))
                nc.vector.scalar_tensor_tensor(
                    t1v, xv[:, :, :, 0, :, 1], sc(K1 + 1), t1v,
                    ALU.mult, ALU.add)
                nc.vector.scalar_tensor_tensor(
                    t1v, xv[:, :, :, 1, :, 0], sc(K1 + 2), t1v,
                    ALU.mult, ALU.add)
                nc.vector.scalar_tensor_tensor(
                    t1v, xv[:, :, :, 1, :, 1], sc(K1 + 3), t1v,
                    ALU.mult, ALU.add)
                nc.scalar.activation(t1[:, :C * 16], t1[:, :C * 16], AF.Relu,
                                     bias=sc(SH1), scale=1.0)

                # conv2 on the 4x4 maps, same shape; f = relu(t2 - 2)
                x1v = t1[:, :C * 16].rearrange(
                    "p (c oh ti ow tj) -> p c oh ti ow tj", oh=2, ti=2, ow=2,
                    tj=2)
                t2 = spool.tile([P, CMAX * 4], F32, tag="t2")
                t2v = t2[:, :C * 4].rearrange("p (c oh ow) -> p c oh ow",
                                              oh=2, ow=2)
                nc.scalar.activation(t2v, x1v[:, :, :, 0, :, 0], AF.Copy,
                                     bias=0.0, scale=sc(K2 + 0))
                nc.vector.scalar_tensor_tensor(
                    t2v, x1v[:, :, :, 0, :, 1], sc(K2 + 1), t2v,
                    ALU.mult, ALU.add)
                nc.vector.scalar_tensor_tensor(
                    t2v, x1v[:, :, :, 1, :, 0], sc(K2 + 2), t2v,
                    ALU.mult, ALU.add)
                nc.vector.scalar_tensor_tensor(
                    t2v, x1v[:, :, :, 1, :, 1], sc(K2 + 3), t2v,
                    ALU.mult, ALU.add)
                nc.scalar.activation(t2[:, :C * 4], t2[:, :C * 4], AF.Relu,
                                     bias=sc(SH2), scale=1.0)
                fv = t2[:, :C * 4].rearrange("p (c i) -> p c i", i=4)

                # out tile layout per sample: [cls0, cls1, raw0, raw1]
                ot = opool.tile([P, CMAX * 4], F32, tag="ot")
                ov = ot[:, :C * 4].rearrange("p (c four) -> p c four", four=4)

                if PE_MLP:
                    # MLP on the tensor engine: transpose f into feature-major
                    # blocks, block-diag matmuls for W1/W2ext (bias folded
                    # into the PSUM->SBUF activation; W2ext also emits
                    # d = raw0-raw1 per sample), transpose back.
                    NF = C * 4
                    nblk = NF // 128
                    fT = ppool.tile([P, CMAX * 4], F32, tag="fT")
                    for k in range(nblk):
                        nc.tensor.matmul(
                            fT[:, k * 128:(k + 1) * 128],
                            t2[:, k * 128:(k + 1) * 128],
                            pc[:, 0:128], start=True, stop=True,
                            is_transpose=True)
                    fTs = tpool.tile([P, CMAX * 4], F32, tag="fTs")
                    nc.scalar.activation(fTs[:, :NF], fT[:, :NF], AF.Copy,
                                         bias=0.0, scale=1.0)
                    hp = ppool.tile([96, CMAX * 4], F32, tag="hp")
                    # matmul outputs must not straddle a 2KB PSUM bank:
                    # chunk at 512-fp32 boundaries (also the fp32 moving max)
                    for s0_ in range(0, NF, 512):
                        w = min(512, NF - s0_)
                        nc.tensor.matmul(
                            hp[:, s0_:s0_ + w], pc[:, 128:224],
                            fTs[:, s0_:s0_ + w], start=True, stop=True)
                    hTs = tpool.tile([96, CMAX * 4], F32, tag="hTs")
                    nc.scalar.activation(hTs[:, :NF], hp[:, :NF], AF.Relu,
                                         bias=pc[:96, 320:321], scale=1.0)
                    rp = ppool.tile([96, CMAX * 4], F32, tag="rp")
                    for s0_ in range(0, NF, 512):
                        w = min(512, NF - s0_)
                        nc.tensor.matmul(
                            rp[:, s0_:s0_ + w], pc[:96, 224:320],
                            hTs[:96, s0_:s0_ + w], start=True, stop=True)
                    rTs = tpool.tile([96, CMAX * 4], F32, tag="rTs")
                    nc.scalar.activation(rTs[:, :NF], rp[:, :NF], AF.Identity,
                                         bias=pc[:96, 321:322], scale=1.0)
                    # back-transpose: each [128, 96] chunk parked in a 512B-
                    # aligned 128-col slot so no output straddles a bank
                    rT = ppool.tile([P, CMAX * 4], F32, tag="rT")
                    for k in range(nblk):
                        nc.tensor.matmul(
                            rT[:, k * 128:k * 128 + 96],
                            rTs[:96, k * 128:(k + 1) * 128],
                            pc[:96, 0:96], start=True, stop=True,
                            is_transpose=True)
                    rTq = rT[:, :nblk * 128].rearrange(
                        "p (k w) -> p k w", w=128)[:, :, 0:96].rearrange(
                        "p k (s j) -> p k s j", j=3)
                    ovq = ov[:, :, 2:4].rearrange(
                        "p (k s) j -> p k s j", k=nblk)
                    nc.scalar.activation(ovq, rTq[:, :, :, 0:2], AF.Copy,
                                         bias=0.0, scale=1.0)
                    cls0 = ov[:, :, 0].rearrange("p (k s) -> p k s", k=nblk)
                    cls1 = ov[:, :, 1].rearrange("p (k s) -> p k s", k=nblk)
                    dv = rTq[:, :, :, 2]
                    nc.scalar.activation(cls0, dv, AF.Sigmoid,
                                         bias=0.0, scale=1.0)
                    nc.scalar.activation(cls1, dv, AF.Sigmoid,
                                         bias=0.0, scale=-1.0)
                    stq.dma_start(out_view, ot[:, :C * 4])
                else:
                    # h_j = relu(W1[j,:] f + b1_j), j-major [3C], relu in place
                    h = spool.tile([P, CMAX * 3], F32, tag="h")
                    for j in range(3):
                        hj = h[:, j * C:(j + 1) * C]
                        nc.scalar.activation(hj, fv[:, :, 0], AF.Identity,
                                             bias=sc(B1C + j),
                                             scale=sc(W1C + 4 * j))
                        for i in range(1, 4):
                            nc.vector.scalar_tensor_tensor(
                                hj, fv[:, :, i], sc(W1C + 4 * j + i), hj,
                                ALU.mult, ALU.add)
                        nc.scalar.activation(hj, hj, AF.Relu, bias=0.0,
                                             scale=1.0)
                    hrv = h[:, :C * 3].rearrange("p (j c) -> p j c", j=3)
                    for j in range(2):
                        rj = ov[:, :, 2 + j]
                        nc.scalar.activation(rj, hrv[:, 0, :], AF.Identity,
                                             bias=sc(B2C + j),
                                             scale=sc(W2C + 3 * j))
                        for i in range(1, 3):
                            nc.vector.scalar_tensor_tensor(
                                rj, hrv[:, i, :], sc(W2C + 3 * j + i), rj,
                                ALU.mult, ALU.add)

                if not PE_MLP:
                    d = spool.tile([P, CMAX], F32, tag="d")
                    nc.vector.tensor_sub(d[:, :C], ov[:, :, 2], ov[:, :, 3])
                    nc.scalar.activation(ov[:, :, 0], d[:, :C], AF.Sigmoid,
                                         bias=0.0, scale=1.0)
                    nc.scalar.activation(ov[:, :, 1], d[:, :C], AF.Sigmoid,
                                         bias=0.0, scale=-1.0)
                    stq.dma_start(out_view, ot[:, :C * 4])

    _split_multiwaits(nc)
    return nc


_NC = None


def _get_nc():
    global _NC
    if _NC is None:
        _NC = _build()
    return _NC


def _pe_consts(W1, b1, W2, b2):
    W1 = np.asarray(W1, dtype=np.float32).reshape(3, 4)
    W2 = np.asarray(W2, dtype=np.float32).reshape(2, 3)
    b1 = np.asarray(b1, dtype=np.float32).reshape(3)
    b2 = np.asarray(b2, dtype=np.float32).reshape(2)
    W2e = np.vstack([W2, W2[0] - W2[1]])                # [3, 3]
    b2e = np.array([b2[0], b2[1], b2[0] - b2[1]], dtype=np.float32)
    pc = np.zeros((P, 322), dtype=np.float32)
    pc[:, 0:128] = np.eye(128, dtype=np.float32)
    pc[:, 128:224] = np.kron(np.eye(32, dtype=np.float32), W1.T)
    pc[:96, 224:320] = np.kron(np.eye(32, dtype=np.float32), W2e.T)
    pc[:96, 320] = np.tile(b1, 32)
    pc[:96, 321] = np.tile(b2e, 32)
    return np.ascontiguousarray(pc)


def _wconst_row(conv1_w, conv2_w, W1, b1, W2, b2):
    row = np.concatenate([
        np.asarray(conv1_w, dtype=np.float32).reshape(4),
        np.asarray(conv2_w, dtype=np.float32).reshape(4),
        np.asarray(W1, dtype=np.float32).reshape(12),
        np.asarray(b1, dtype=np.float32).reshape(3),
        np.asarray(W2, dtype=np.float32).reshape(6),
        np.asarray(b2, dtype=np.float32).reshape(2),
        np.array([-0.2, -2.0], dtype=np.float32),
    ])
    assert row.shape[0] == NW
    return row


TIMED_REPS = 32


def _timed(np_inputs, iters=16, reps=TIMED_REPS):
    """Measure steady-state per-pass HW time.

    Builds a timing variant of the kernel that repeats the full pipeline
    `reps` times inside one NEFF execution (re-reading the same HBM input),
    so device time per call (~reps * pass) dwarfs host dispatch (~1 ms) and
    the axon sync overhead (~75 ms) cancels in a two-burst slope. Calls are
    serialized by donation-chaining the output buffer.
    """
    import time

    import jax
    import jax.core
    import jax.numpy as jnp
    from jax.experimental.shard_map import shard_map
    from jax.sharding import Mesh, NamedSharding, PartitionSpec

    from concourse import bass2jax as b2j

    x = np.ascontiguousarray(
        np.asarray(np_inputs["x"], dtype=np.float32).reshape(B, 64))
    row = _wconst_row(np_inputs["conv1_w"], np_inputs["conv2_w"],
                      np_inputs["W1"], np_inputs["b1"], np_inputs["W2"],
                      np_inputs["b2"])
    wconst = np.ascontiguousarray(
        np.tile(row[None, :], (P * N_CORES, 1)).astype(np.float32))
    pcst = np.ascontiguousarray(np.tile(
        _pe_consts(np_inputs["W1"], np_inputs["b1"], np_inputs["W2"],
                   np_inputs["b2"]), (N_CORES, 1)))

    nc = _build(reps=reps)
    b2j.install_neuronx_cc_hook()
    devices = jax.devices()[:N_CORES]
    mesh = Mesh(np.asarray(devices), ("core",))
    spec = PartitionSpec("core")
    sh = NamedSharding(mesh, spec)
    out_aval = jax.core.ShapedArray((BC, 4), jnp.float32)

    def _body(xs, ws, ps, zs):
        outs = b2j._bass_exec_p.bind(
            xs, ws, ps, zs, b2j.partition_id_tensor(),
            out_avals=(out_aval,),
            in_names=("x", "wconst", "pconst", "out", "partition_id"),
            out_names=("out",),
            lowering_input_output_aliases=(),
            sim_require_finite=True,
            sim_require_nnan=True,
            nc=nc,
        )
        return outs[0]

    fn = jax.jit(
        shard_map(_body, mesh=mesh, in_specs=(spec, spec, spec, spec),
                  out_specs=spec, check_rep=False),
        donate_argnums=(3,), keep_unused=True)

    X = jax.device_put(x, sh)
    W = jax.device_put(wconst, sh)
    PC = jax.device_put(pcst, sh)
    X.block_until_ready()
    W.block_until_ready()
    PC.block_until_ready()

    z = fn(X, W, PC, np.zeros((B, 4), np.float32))
    z.block_until_ready()  # compile + warm

    def run_n(n, z):
        t0 = time.perf_counter()
        for _ in range(n):
            z = fn(X, W, PC, z)
        z.block_until_ready()
        return time.perf_counter() - t0, z

    base = 2
    slopes = []
    for _ in range(5):
        t1, z = run_n(base, z)
        t2, z = run_n(base + iters, z)
        slopes.append((t2 - t1) / iters)
    slopes.sort()
    if os.environ.get("TIMED_VERBOSE"):
        print("slopes/pass us:",
              [f"{s / reps * 1e6:.1f}" for s in slopes], flush=True)
    ns = slopes[len(slopes) // 2] / reps * 1e9
    return ns, np.asarray(z)


def kernel(x, conv1_w, conv2_w, W1, b1, W2, b2):
    x = np.ascontiguousarray(np.asarray(x, dtype=np.float32)).reshape(B, 64)
    row = _wconst_row(conv1_w, conv2_w, W1, b1, W2, b2)
    wconst = np.ascontiguousarray(np.tile(row[None, :], (P, 1)))

    nc = _get_nc()
    pcst = _pe_consts(W1, b1, W2, b2)
    in_maps = [
        {"x": np.ascontiguousarray(x[i * BC:(i + 1) * BC]), "wconst": wconst,
         "pconst": pcst}
        for i in range(N_CORES)
    ]
    res = run_bass_kernel_spmd(nc, in_maps, core_ids=list(range(N_CORES)))
    out = np.concatenate([res.results[i]["out"] for i in range(N_CORES)], axis=0)
    classification = np.ascontiguousarray(out[:, 0:2])
    raw = np.ascontiguousarray(out[:, 2:4])
    return classification, raw
